# revision 8
# baseline (speedup 1.0000x reference)
"""GATv2 layer kernel for Trainium2 (8 NeuronCores, SPMD).

Math note: in the reference, the per-edge value vectors are gathered from the
*destination* node (Vv = V[dest] @ Wv^T + bv) and the scatter-softmax is also
grouped by destination. Within a destination segment Vv is constant, and the
softmax weights sum to 1, so

    H[n] = (V[n] @ Wv_w^T + Wv_b) * [n has >= 1 incoming edge]

exactly (up to f32 rounding of the softmax-weight sum, ~1e-7 relative).

Sharding: nodes are partitioned contiguously across the 8 cores (edge data is
not needed on device at all); the small [128,128] weight is replicated. Each
core computes H^T = Wv @ V_shard^T on the tensor engine in fp16 (the 2e-2
rel-err budget is ~10x what fp16 costs), with the bias folded into the
PSUM->SBUF downcast as a per-partition scalar add, alternating between the
DVE and ACT engines so neither is the serial bottleneck. The transposed
layout makes the weight the PE-stationary operand and gives both DMA
directions long contiguous per-partition lines. The incoming-edge mask is
applied on host (the in-degree histogram is host-side sharding prep either
way, and with E/N = 12.5 it almost always masks nothing).
"""

import os

import numpy as np

import concourse.bacc as bacc
import concourse.bass as bass  # noqa: F401  (kept for parity with bacc deps)
import concourse.mybir as mybir
import concourse.tile as tile
from concourse.bass_utils import run_bass_kernel_spmd

N_CORES = 8
P = 128
D = 128
MM_N = 512  # matmul free dim = one PSUM bank of f32

_module_cache = {}


def _build_module(n_cols, chunk, psbufs, repeat=1, out_eng="sync",
                  const_eng="sync", in_split=1, act_every=0, warm_act=False,
                  first_small=False, in_engs=None, out_engs=None,
                  cast_pattern=None, const_engs=None, out_tail_eng=None):
    """One SPMD NeuronCore program: hT = wvT^T @ vT + b  ([D, n_cols] fp16).

    n_cols: node count per core (any size; last matmul subtile may be <MM_N).
    chunk: columns per DMA chunk (multiple of MM_N).
    repeat: >1 wraps the body in a hardware loop (timing rigs only).
    out_eng/const_eng: engine issuing output/const DMAs ("sync", "scalar",
        "gpsimd", "vector", "tensor").
    in_split: input chunk DMAs round-robin over this many issue engines.
    """
    f32 = mybir.dt.float32
    f16 = mybir.dt.float16
    assert chunk % MM_N == 0

    nc = bacc.Bacc("TRN2", target_bir_lowering=False, debug=False)
    vT_in = nc.dram_tensor("vT", [D, n_cols], f16, kind="ExternalInput")
    wvT_in = nc.dram_tensor("wvT", [D, D], f16, kind="ExternalInput")
    b_in = nc.dram_tensor("b", [D, 1], f32, kind="ExternalInput")
    hT_out = nc.dram_tensor("hT", [D, n_cols], f16, kind="ExternalOutput")

    eng = lambda name: getattr(nc, name)
    if in_engs is None:
        in_engs = ["sync", "gpsimd", "scalar", "tensor"][:in_split]
    if out_engs is None:
        out_engs = [out_eng]

    chunks = []
    c0 = 0
    if first_small:
        chunks.append((0, MM_N))
        c0 = MM_N
    while c0 < n_cols:
        chunks.append((c0, min(chunk, n_cols - c0)))
        c0 += chunk
    n_chunks = len(chunks)

    with tile.TileContext(nc) as tc:
        with (
            tc.tile_pool(name="const", bufs=1) as cpool,
            tc.tile_pool(name="vin", bufs=n_chunks) as vpool,
            tc.tile_pool(name="hout", bufs=n_chunks) as hpool,
            tc.tile_pool(name="ps", bufs=psbufs, space="PSUM") as pspool,
        ):
            ce = const_engs or [const_eng, const_eng]
            wvT_sb = cpool.tile([D, D], f16)
            eng(ce[0]).dma_start(out=wvT_sb[:], in_=wvT_in[:])
            b_sb = cpool.tile([D, 1], f32)
            eng(ce[1]).dma_start(out=b_sb[:], in_=b_in[:])
            if act_every and warm_act:
                # touch the Identity activation table so LoadActFuncSet
                # overlaps the input DMA stream instead of the first cast
                warm = cpool.tile([1, 1], f32)
                nc.vector.memset(warm[:], 0.0)
                nc.scalar.activation(
                    out=warm[:], in_=warm[:],
                    func=mybir.ActivationFunctionType.Identity,
                )

            def body():
                mm_i = 0
                for ci, (c0, w) in enumerate(chunks):
                    v_sb = vpool.tile([D, chunk], f16, tag="v")
                    eng(in_engs[ci % len(in_engs)]).dma_start(
                        out=v_sb[:, :w], in_=vT_in[:, c0 : c0 + w]
                    )
                    h_sb = hpool.tile([D, chunk], f16, tag="h")
                    for s0 in range(0, w, MM_N):
                        sw = min(MM_N, w - s0)
                        assert sw > 0
                        ps = pspool.tile([P, MM_N], f32, tag="ps")
                        nc.tensor.matmul(
                            out=ps[:, :sw],
                            lhsT=wvT_sb[:],
                            rhs=v_sb[:, s0 : s0 + sw],
                            start=True,
                            stop=True,
                        )
                        mm_i += 1
                        if cast_pattern:
                            on_act = cast_pattern[(mm_i - 1) % len(cast_pattern)] == "a"
                        else:
                            on_act = act_every and mm_i % act_every == 0
                        if on_act:
                            nc.scalar.activation(
                                out=h_sb[:, s0 : s0 + sw],
                                in_=ps[:, :sw],
                                func=mybir.ActivationFunctionType.Identity,
                                bias=b_sb[:, 0:1],
                            )
                        else:
                            nc.vector.tensor_scalar_add(
                                out=h_sb[:, s0 : s0 + sw],
                                in0=ps[:, :sw],
                                scalar1=b_sb[:, 0:1],
                            )
                    oe = out_engs[ci % len(out_engs)]
                    if out_tail_eng and ci == n_chunks - 1:
                        # SWDGE completion sems arrive ~1.7us after engine
                        # issue; route the final store via HWDGE so the
                        # epilogue drain isn't held hostage by it
                        oe = out_tail_eng
                    eng(oe).dma_start(
                        out=hT_out[:, c0 : c0 + w], in_=h_sb[:, :w]
                    )

            if repeat > 1:
                with tc.For_i(0, repeat):
                    body()
            else:
                body()

    nc.compile()
    return nc


# Structure tuned on the CoreSim cost model, chunking and engine pattern
# picked by interleaved For_i-slope A/B on hardware (chunk 3072 beat 1024 by
# ~2us/iter consistently; 4096+ collapses pipelining): weight + input stream
# on SP (HWDGE), first output chunk on gpsimd (SWDGE completion lag hides
# mid-stream), later outputs on SP/ACT once their streams free up,
# PSUM->SBUF downcasts alternating DVE/ACT, ACT's Identity table pre-warmed,
# and a small first chunk so the output stream starts early.
_OUT_ENGS = ["gpsimd", "sync", "scalar"]


def _get_module(n_cols, chunk, psbufs):
    key = (n_cols, chunk, psbufs)
    if key not in _module_cache:
        _module_cache[key] = _build_module(
            n_cols, chunk, psbufs, act_every=2, warm_act=True,
            first_small=True, in_engs=["sync"], out_engs=_OUT_ENGS,
            const_engs=["sync", "gpsimd"],
        )
    return _module_cache[key]


def kernel(V, E, edge_index, Wq_w, Wq_b, Wk_w, Wk_b, Wv_w, Wv_b, We_w, We_b,
           a_w, a_b, _trace=False):
    V = np.asarray(V)
    n_nodes, d = V.shape
    assert d == D and n_nodes % N_CORES == 0
    npc = n_nodes // N_CORES                  # nodes per core
    n_cols = npc                              # exact size, no pad transfer
    chunk = int(os.environ.get("K2_CHUNK", "3072"))
    psbufs = int(os.environ.get("K2_PSBUFS", "8"))

    dest = np.asarray(edge_index)[1]
    counts = np.bincount(dest, minlength=n_nodes)

    V16 = np.asarray(V, dtype=np.float16)
    wvT = np.ascontiguousarray(np.asarray(Wv_w, dtype=np.float16).T)
    bcol = np.ascontiguousarray(np.asarray(Wv_b, dtype=np.float32)[:, None])

    in_maps = []
    for c in range(N_CORES):
        vpT = np.ascontiguousarray(V16[c * npc : (c + 1) * npc].T)
        in_maps.append({"vT": vpT, "wvT": wvT, "b": bcol})

    nc = _get_module(n_cols, chunk, psbufs)
    try:
        res = run_bass_kernel_spmd(nc, in_maps, core_ids=list(range(N_CORES)),
                                   trace=_trace)
    except ModuleNotFoundError:
        # axon NTFF profiling hook unavailable in this environment
        res = run_bass_kernel_spmd(nc, in_maps, core_ids=list(range(N_CORES)),
                                   trace=False)
    out = np.concatenate(
        [res.results[c]["hT"][:, :npc].T for c in range(N_CORES)], axis=0
    ).astype(np.float32)
    # zero-in-degree nodes receive an empty segment-sum in the reference
    out[counts == 0] = 0.0
    if _trace:
        return out, res
    return out
